# revision 2
# baseline (speedup 1.0000x reference)
"""Trainium2 Bass kernel for nn_BartCrossAttention (B=4, L=1024, D=1024, H=16, HD=64).

Sharding: 8 cores; core c handles query tokens [512c, 512c+512) (batch b = c//2).
Each core recomputes K/V projections for its *whole* batch (1024 kv tokens) so no
collective is needed; the host slices inputs per core and concatenates outputs.

v2 dataflow (all matmul operands bf16; host pre-casts/pre-permutes):
  - activations (hid, kv) arrive bf16 token-major; XBAR DMA-transpose loads them
    straight into feature-major SBUF tiles (no PE transposes, no PSUM evictions)
  - weights stored [p][chunk][col] in DRAM so each full weight is one 128x16KB
    contiguous DMA; all four weights stay SBUF-resident
  - V proj -> v65 (V plus a ones column per head for fused softmax denominators)
  - per head-pair hp: K^T/Q^T projections (Wq pre-scaled 1/8), then per kv tile:
    S^T = K^T_h.T @ Q^T_h; attn = exp(S^T) (scores O(9), exp safe in fp32 PSUM);
    matmul(lhsT=[V_h|1], rhs=attn) accumulated -> rows 0..63 ctx^T, row 64 sums.
    attnV lags scores by one tile so the EXP latency is hidden.
  - normalization pipelined per head: reciprocal of the sums row + gpsimd
    partition_broadcast + multiply, overlapped with the next head's attention
  - epilogue: out = ctxT_tile.T @ Wo + out_bias
"""
import sys

for _p in ("/opt/trn_rl_repo",):
    if _p not in sys.path:
        sys.path.insert(0, _p)

import numpy as np
import ml_dtypes

import concourse.bass as bass
import concourse.mybir as mybir
import concourse.tile as tile
from concourse import bacc
import concourse.bass_utils as bass_utils

F32 = mybir.dt.float32
BF16 = mybir.dt.bfloat16

P = 128
D = 1024        # model dim
H = 16          # heads
NCORES = 8
TQ = 512        # query tokens per core
LK = 1024       # kv tokens per batch
B, LQ = 4, 1024

_CACHE = {}


def _build_core_program():
    nc = bacc.Bacc("TRN2", target_bir_lowering=False, debug=False,
                   num_devices=NCORES)

    hid_s = nc.dram_tensor("hid_s", [TQ, D], BF16, kind="ExternalInput")
    kv_s = nc.dram_tensor("kv_s", [LK, D], BF16, kind="ExternalInput")
    wq_d = nc.dram_tensor("wq_d", [P, 8 * D], BF16, kind="ExternalInput")
    wk_d = nc.dram_tensor("wk_d", [P, 8 * D], BF16, kind="ExternalInput")
    wv_d = nc.dram_tensor("wv_d", [P, 8 * D], BF16, kind="ExternalInput")
    wo_d = nc.dram_tensor("wo_d", [P, 8 * D], BF16, kind="ExternalInput")
    qb_d = nc.dram_tensor("qb", [D], F32, kind="ExternalInput")
    kb_d = nc.dram_tensor("kb", [D], F32, kind="ExternalInput")
    vb_d = nc.dram_tensor("vb", [D], F32, kind="ExternalInput")
    ob_d = nc.dram_tensor("ob", [D], F32, kind="ExternalInput")
    out_s = nc.dram_tensor("out_s", [TQ, D], F32, kind="ExternalOutput")

    Exp = mybir.ActivationFunctionType.Exp
    add = mybir.AluOpType.add
    mult = mybir.AluOpType.mult

    with tile.TileContext(nc) as tc:
        with (
            tc.tile_pool(name="setup", bufs=1) as setup,
            tc.tile_pool(name="wpool", bufs=1) as wpool,
            tc.tile_pool(name="big", bufs=1) as big,
            tc.tile_pool(name="attn", bufs=2) as attnp,
            tc.tile_pool(name="rb", bufs=2) as rbp,
            tc.tile_pool(name="outp", bufs=2) as outp,
            tc.tile_pool(name="psmm", bufs=2, space="PSUM") as psmm,
            tc.tile_pool(name="pssc2", bufs=2, space="PSUM") as pssc2,
            tc.tile_pool(name="psctx", bufs=2, space="PSUM") as psctx,
        ):
            # ---- persistent tiles ----
            kvT = big.tile([P, 8, LK], BF16, tag="kvT")      # kv^T [1024, 1024]
            hidT = big.tile([P, 8, TQ], BF16, tag="hidT")    # hid^T [1024, 512]
            KT = big.tile([P, 8, LK], BF16, tag="KT")        # K^T per pair
            v65 = big.tile([P, 8, H * 65], BF16, tag="v65")  # V+ones columns
            qT = big.tile([P, 8, TQ], BF16, tag="qT")        # Q^T per pair
            ctxT = big.tile([P, 8, TQ], BF16, tag="ctxT")    # normalized ctx^T

            def t_in(dst8, src_dram, tq):
                # transpose-load token range [256*tq, 256*tq+256) of all 8
                # feature chunks: dst[p, dc, tok] = src[tok, dc*128+p]
                for dc in range(8):
                    nc.sync.dma_start(
                        dst8[:, dc, tq * 256:(tq + 1) * 256],
                        src_dram.ap()[tq * 256:(tq + 1) * 256,
                                      dc * P:(dc + 1) * P],
                        transpose=True,
                    )

            def load_w(dram, tag):
                t = wpool.tile([P, 8, D], BF16, tag=tag)
                nc.sync.dma_start(
                    t[:], dram.ap().rearrange("p (dd o) -> p dd o", dd=8))
                return t

            # DMA order: kv tokens 0-255 first (feeds V proj tile 0), then wv,
            # rest of kv, hid, remaining weights.
            t_in(kvT, kv_s, 0)
            wv_t = load_w(wv_d, "wv")
            for tq in range(1, 4):
                t_in(kvT, kv_s, tq)
            for tq in range(2):
                t_in(hidT, hid_s, tq)
            wk_t = load_w(wk_d, "wk")
            wq_t = load_w(wq_d, "wq")
            wo_t = load_w(wo_d, "wo")

            # ---- biases ----
            qb_sb = setup.tile([P, 8], F32, tag="qb")
            nc.sync.dma_start(qb_sb[:], qb_d.ap().rearrange("(o p) -> p o", p=P))
            kb_sb = setup.tile([P, 8], F32, tag="kb")
            nc.sync.dma_start(kb_sb[:], kb_d.ap().rearrange("(o p) -> p o", p=P))
            vbB = setup.tile([P, D], F32, tag="vbB")
            vb_row = setup.tile([1, D], F32, tag="vb_row")
            nc.sync.dma_start(vb_row[:], vb_d.ap()[None, :])
            nc.gpsimd.partition_broadcast(vbB[:], vb_row[:])
            obB = setup.tile([P, D], F32, tag="obB")
            ob_row = setup.tile([1, D], F32, tag="ob_row")
            nc.sync.dma_start(ob_row[:], ob_d.ap()[None, :])
            nc.gpsimd.partition_broadcast(obB[:], ob_row[:])

            # ones columns of v65 (col 64 of each head block)
            onesF = setup.tile([P, P], F32, tag="onesF")
            nc.gpsimd.memset(onesF[:], 1.0)
            nc.vector.tensor_copy(
                v65[:].rearrange("p t (h x) -> p t h x", x=65)[:, :, :, 64:65],
                onesF[:].rearrange("p (t h x) -> p t h x", t=8, h=16))

            # ---- V projection ----
            for half in range(2):
                for ti in range(8):
                    pp = psmm.tile([P, 512], F32, tag="pp")
                    for di in range(8):
                        nc.tensor.matmul(
                            pp[:],
                            kvT[:, di, ti * P:(ti + 1) * P],
                            wv_t[:, di, half * 512:(half + 1) * 512],
                            start=(di == 0), stop=(di == 7),
                        )
                    dst = v65[:].rearrange(
                        "p t (h x) -> p t h x", x=65)[
                        :, ti, half * 8:(half + 1) * 8, 0:64]
                    nc.vector.tensor_tensor(
                        dst, pp[:],
                        vbB[:, half * 512:(half + 1) * 512], add)

            # ---- main loop: per head-pair K/Q projection + attention ----
            def emit_kproj(hp, nk):
                pp = psmm.tile([P, 512], F32, tag="pp", name=f"ppk{hp}_{nk}")
                for di in range(8):
                    nc.tensor.matmul(
                        pp[:],
                        wk_t[:, di, hp * P:(hp + 1) * P],
                        kvT[:, di, nk * 512:(nk + 1) * 512],
                        start=(di == 0), stop=(di == 7),
                    )
                nc.vector.tensor_scalar(
                    KT[:, hp, nk * 512:(nk + 1) * 512], pp[:],
                    kb_sb[:, hp:hp + 1], None, add)

            def emit_qproj(hp):
                pq = psmm.tile([P, 512], F32, tag="pp", name=f"ppq{hp}")
                for di in range(8):
                    nc.tensor.matmul(
                        pq[:],
                        wq_t[:, di, hp * P:(hp + 1) * P],
                        hidT[:, di, :],
                        start=(di == 0), stop=(di == 7),
                    )
                nc.vector.tensor_scalar(qT[:, hp, :], pq[:],
                                        qb_sb[:, hp:hp + 1], None, add)

            def emit_scores(hp, t):
                sc2 = pssc2.tile([P, 1024], F32, tag="sc2",
                                 name=f"sc2_{hp}_{t}")
                for hh in range(2):
                    lo = 64 * hh
                    nc.tensor.matmul(
                        sc2[:, hh * 512:(hh + 1) * 512],
                        KT[lo:lo + 64, hp, t * P:(t + 1) * P],
                        qT[lo:lo + 64, hp, :],
                        start=True, stop=True,
                    )
                at2 = attnp.tile([P, 1024], BF16, tag="at")
                nc.scalar.activation(at2[:], sc2[:], Exp)
                return at2

            def emit_attnv(hp, t, at2, ctx_ps):
                for hh in range(2):
                    h = 2 * hp + hh
                    nc.tensor.matmul(
                        ctx_ps[hh][:],
                        v65[:, t, h * 65:(h + 1) * 65],
                        at2[:, hh * 512:(hh + 1) * 512],
                        start=(t == 0), stop=(t == 7),
                    )

            def emit_norm(hp, ctx_ps):
                # ctx_ps[hh]: rows 0..63 unnormalized ctx^T, row 64 sums
                for hh in range(2):
                    rcp = rbp.tile([1, 512], F32, tag="rcp",
                                   name=f"rcp{hp}_{hh}")
                    nc.vector.reciprocal(rcp[:], ctx_ps[hh][64:65, :])
                    bc = rbp.tile([P, 512], F32, tag="bc",
                                  name=f"bc{hp}_{hh}")
                    nc.gpsimd.partition_broadcast(bc[:], rcp[:])
                    lo = 64 * hh
                    nc.vector.tensor_tensor(
                        ctxT[lo:lo + 64, hp, :], ctx_ps[hh][0:64, :],
                        bc[lo:lo + 64, :], mult)

            emit_kproj(0, 0)
            emit_kproj(0, 1)
            emit_qproj(0)

            for hp in range(8):
                nxt = hp + 1
                ctx_ps = [psctx.tile([65, 512], F32, tag="ctx",
                                     name=f"ctx{hp}_{i}")
                          for i in range(2)]
                # software pipeline: attnV lags scores by one kv tile so the
                # EXP of tile t runs while the PE does scores(t+1)/attnV(t-1)
                at_prev = emit_scores(hp, 0)
                for t in range(1, 8):
                    at_cur = emit_scores(hp, t)
                    emit_attnv(hp, t - 1, at_prev, ctx_ps)
                    at_prev = at_cur
                    if nxt < 8:
                        if t == 2:
                            emit_kproj(nxt, 0)
                        elif t == 4:
                            emit_kproj(nxt, 1)
                        elif t == 6:
                            emit_qproj(nxt)
                emit_attnv(hp, 7, at_prev, ctx_ps)
                emit_norm(hp, ctx_ps)

            # ---- epilogue: out projection ----
            for half in range(2):
                for mi in range(4):
                    po = psmm.tile([P, 512], F32, tag="pp")
                    for fj in range(8):
                        nc.tensor.matmul(
                            po[:],
                            ctxT[:, fj, mi * P:(mi + 1) * P],
                            wo_t[:, fj, half * 512:(half + 1) * 512],
                            start=(fj == 0), stop=(fj == 7),
                        )
                    ot = outp.tile([P, 512], F32, tag="ot")
                    nc.vector.tensor_tensor(
                        ot[:], po[:],
                        obB[:, half * 512:(half + 1) * 512], add)
                    nc.sync.dma_start(
                        out_s.ap().rearrange("(mm p) d -> p mm d", p=P)[
                            :, mi, half * 512:(half + 1) * 512],
                        ot[:])

    nc.compile()
    return nc


def _w_layout(w_t):
    # [D, D] weight (already transposed: rows = contraction dim) ->
    # [128, 8*D] bf16 with element (p, dd*D+o) = w_t[dd*128+p, o]
    return np.ascontiguousarray(
        w_t.reshape(8, P, D).transpose(1, 0, 2).reshape(P, 8 * D)
    ).astype(ml_dtypes.bfloat16)


def _prep_inputs(hidden_states, key_value_states, q_weight, q_bias,
                 kv_weight, kv_bias, out_weight, out_bias):
    f32 = np.float32
    bf16 = ml_dtypes.bfloat16
    hid = np.asarray(hidden_states, f32).reshape(B * LQ, D).astype(bf16)
    kv = np.asarray(key_value_states, f32).reshape(B * LK, D).astype(bf16)
    scale = f32(1.0 / 8.0)

    # de-interleave kv rows: row e <-> (h=e//128, j=(e%128)//64, d=e%64)
    e = np.arange(2 * D)
    kmask = (e % 128) < 64
    kidx, vidx = e[kmask], e[~kmask]
    kvw = np.asarray(kv_weight, f32)
    kvb = np.asarray(kv_bias, f32)

    shared = {
        "wq_d": _w_layout((np.asarray(q_weight, f32) * scale).T),
        "wk_d": _w_layout(np.ascontiguousarray(kvw[kidx].T)),
        "wv_d": _w_layout(np.ascontiguousarray(kvw[vidx].T)),
        "wo_d": _w_layout(np.asarray(out_weight, f32).T),
        "qb": np.ascontiguousarray(np.asarray(q_bias, f32) * scale),
        "kb": np.ascontiguousarray(kvb[kidx]),
        "vb": np.ascontiguousarray(kvb[vidx]),
        "ob": np.ascontiguousarray(np.asarray(out_bias, f32)),
    }
    in_maps = []
    for c in range(NCORES):
        b = c // 2
        m = dict(shared)
        m["hid_s"] = np.ascontiguousarray(hid[c * TQ:(c + 1) * TQ])
        m["kv_s"] = np.ascontiguousarray(kv[b * LK:(b + 1) * LK])
        in_maps.append(m)
    return in_maps


def kernel(hidden_states, key_value_states, q_weight, q_bias,
           kv_weight, kv_bias, out_weight, out_bias, _trace=False):
    if "nc" not in _CACHE:
        _CACHE["nc"] = _build_core_program()
    nc = _CACHE["nc"]
    in_maps = _prep_inputs(hidden_states, key_value_states, q_weight, q_bias,
                           kv_weight, kv_bias, out_weight, out_bias)
    res = bass_utils.run_bass_kernel_spmd(
        nc, in_maps, core_ids=list(range(NCORES)), trace=_trace)
    _CACHE["last_result"] = res
    out = np.concatenate([r["out_s"] for r in res.results], axis=0)
    return out.reshape(B, LQ, D)


# revision 7
# speedup vs baseline: 1.7313x; 1.7313x over previous
"""Trainium2 Bass kernel for nn_BartCrossAttention (B=4, L=1024, D=1024, H=16, HD=64).

Sharding: 8 cores; core c handles query tokens [512c, 512c+512) (batch b = c//2).
Each core recomputes K/V projections for its *whole* batch (1024 kv tokens) so no
collective is needed; the host slices inputs per core and concatenates outputs.

v3 dataflow (all matmul operands bf16; host pre-casts/pre-permutes):
  - activations (hid, kv) arrive bf16 token-major, loaded with plain DMAs and
    PE-transposed (bf16 transpose = 1 cyc/row, ~53ns per 128x128 block) into
    feature-major tiles; evictions alternate scalar/vector engines
  - weights stored [p][chunk][col] in DRAM, loaded in 4 DMA chunks each so the
    descriptors spread across queues; all weights SBUF-resident
  - V proj -> v65 (V plus a ones column per head for fused softmax denominators)
  - per head-pair hp: K^T/Q^T projections (Wq pre-scaled 1/8), then per kv tile:
    S^T = K^T_h.T @ Q^T_h; attn = exp(S^T) (scores O(9), exp safe in fp32 PSUM);
    matmul(lhsT=[V_h|1], rhs=attn) accumulated -> rows 0..63 ctx^T, row 64 sums.
    attnV lags scores by one tile so the EXP latency is hidden.
  - normalization pipelined per head: reciprocal_approx_fast of the sums row
    (DVE, ~5x faster than reciprocal()) + gpsimd partition_broadcast + multiply,
    overlapped with the next head's attention
  - epilogue: out = ctxT_tile.T @ Wo + out_bias
"""
import sys

for _p in ("/opt/trn_rl_repo",):
    if _p not in sys.path:
        sys.path.insert(0, _p)

import numpy as np
import ml_dtypes

import concourse.bass as bass
import concourse.mybir as mybir
import concourse.tile as tile
from concourse import bacc
import concourse.bass_utils as bass_utils
from concourse.masks import make_identity

F32 = mybir.dt.float32
BF16 = mybir.dt.bfloat16

P = 128
D = 1024        # model dim
H = 16          # heads
NCORES = 8
TQ = 512        # query tokens per core
LK = 1024       # kv tokens per batch
B, LQ = 4, 1024

_CACHE = {}


def _build_core_program():
    nc = bacc.Bacc("TRN2", target_bir_lowering=False, debug=False,
                   num_devices=NCORES)

    hid_s = nc.dram_tensor("hid_s", [TQ, D], BF16, kind="ExternalInput")
    kv_s = nc.dram_tensor("kv_s", [LK, D], BF16, kind="ExternalInput")
    wq_d = nc.dram_tensor("wq_d", [P, 8 * D], BF16, kind="ExternalInput")
    wk_d = nc.dram_tensor("wk_d", [P, 8 * D], BF16, kind="ExternalInput")
    wv_d = nc.dram_tensor("wv_d", [P, 8 * D], BF16, kind="ExternalInput")
    wo_d = nc.dram_tensor("wo_d", [P, 8 * D], BF16, kind="ExternalInput")
    qb_d = nc.dram_tensor("qb", [D], F32, kind="ExternalInput")
    kb_d = nc.dram_tensor("kb", [D], F32, kind="ExternalInput")
    vb_d = nc.dram_tensor("vb", [D], F32, kind="ExternalInput")
    ob_d = nc.dram_tensor("ob", [D], F32, kind="ExternalInput")
    out_s = nc.dram_tensor("out_s", [TQ, D], F32, kind="ExternalOutput")

    Exp = mybir.ActivationFunctionType.Exp
    Ident = mybir.ActivationFunctionType.Identity
    add = mybir.AluOpType.add
    mult = mybir.AluOpType.mult

    with tile.TileContext(nc) as tc:
        with (
            tc.tile_pool(name="setup", bufs=1) as setup,
            tc.tile_pool(name="wpool", bufs=1) as wpool,
            tc.tile_pool(name="big", bufs=1) as big,
            tc.tile_pool(name="outp", bufs=2) as outp,
            tc.tile_pool(name="psmm", bufs=2, space="PSUM") as psmm,
        ):
            # ---- persistent tiles ----
            kvT = big.tile([P, 8, LK], BF16, tag="kvT")      # kv^T [1024, 1024]
            hidT = big.tile([P, 8, TQ], BF16, tag="hidT")    # hid^T [1024, 512]
            KT = big.tile([P, 8, LK], BF16, tag="KT")        # K^T per pair
            v65 = big.tile([P, 8, H * 65], BF16, tag="v65")  # V+ones columns
            qT = big.tile([P, 8, TQ], BF16, tag="qT")        # Q^T per pair
            ctxT = big.tile([P, 8, TQ], BF16, tag="ctxT")    # normalized ctx^T

            def load_w(dram, tag):
                # 4 DMA chunks so descriptors spread across queues
                t = wpool.tile([P, 8, D], BF16, tag=tag)
                src = dram.ap().rearrange("p (dd o) -> p dd o", dd=8)
                for qq in range(4):
                    nc.sync.dma_start(t[:, 2 * qq:2 * qq + 2, :],
                                      src[:, 2 * qq:2 * qq + 2, :])
                return t

            identF = setup.tile([P, P], F32, tag="identF")
            make_identity(nc, identF[:])
            ident = setup.tile([P, P], BF16, tag="ident")
            nc.vector.tensor_copy(ident[:], identF[:])

            # ---- biases ----
            qb_sb = setup.tile([P, 8], F32, tag="qb")
            nc.sync.dma_start(qb_sb[:], qb_d.ap().rearrange("(o p) -> p o", p=P))
            kb_sb = setup.tile([P, 8], F32, tag="kb")
            nc.sync.dma_start(kb_sb[:], kb_d.ap().rearrange("(o p) -> p o", p=P))
            vbB = setup.tile([P, D], F32, tag="vbB")
            vb_row = setup.tile([1, D], F32, tag="vb_row")
            nc.sync.dma_start(vb_row[:], vb_d.ap()[None, :])
            nc.gpsimd.partition_broadcast(vbB[:], vb_row[:])
            obB = setup.tile([P, D], F32, tag="obB")
            ob_row = setup.tile([1, D], F32, tag="ob_row")
            nc.sync.dma_start(ob_row[:], ob_d.ap()[None, :])
            nc.gpsimd.partition_broadcast(obB[:], ob_row[:])

            # ones columns of v65 (col 64 of each head block)
            onesF = setup.tile([P, P], F32, tag="onesF")
            nc.gpsimd.memset(onesF[:], 1.0)
            nc.vector.tensor_copy(
                v65[:].rearrange("p t (h x) -> p t h x", x=65)[:, :, :, 64:65],
                onesF[:].rearrange("p (t h x) -> p t h x", t=8, h=16))

            # ---- prologue: stage, transpose, V projection ----
            with (
                tc.tile_pool(name="stg", bufs=8) as stg,
                tc.tile_pool(name="stgh", bufs=4) as stgh,
                tc.tile_pool(name="pstp", bufs=2, space="PSUM") as pstp,
            ):
                # order: kv rows first (feeds transposes+V proj), weights next
                kv_tiles = []
                for tt in range(8):
                    nsrc = stg.tile([P, D], BF16, tag="nsrc",
                                    name=f"kvrow{tt}")
                    nc.sync.dma_start(
                        nsrc[:],
                        kv_s.ap().rearrange("(tt p) d -> p tt d", p=P)[:, tt, :])
                    kv_tiles.append(nsrc)
                    if tt == 1:
                        wv_t = load_w(wv_d, "wv")
                for tt in range(8):
                    nsrc = kv_tiles[tt]
                    for dhalf in range(2):
                        tp = pstp.tile([P, 512], BF16, tag="tp")
                        for q in range(4):
                            di = dhalf * 4 + q
                            nc.tensor.transpose(
                                tp[:, q * P:(q + 1) * P],
                                nsrc[:, di * P:(di + 1) * P],
                                ident[:],
                            )
                        dst_ap = kvT[:, dhalf * 4:dhalf * 4 + 4,
                                     tt * P:(tt + 1) * P]
                        src_ap = tp[:].rearrange("p (q x) -> p q x", q=4)
                        if dhalf == 0:
                            nc.scalar.activation(dst_ap, src_ap, Ident)
                        else:
                            nc.vector.tensor_copy(dst_ap, src_ap)

                # queue hid rows + remaining weights while V proj runs
                hid_tiles = []
                for tt in range(4):
                    nsrc = stgh.tile([P, D], BF16, tag="nsrch",
                                     name=f"hidrow{tt}")
                    nc.sync.dma_start(
                        nsrc[:],
                        hid_s.ap().rearrange("(tt p) d -> p tt d", p=P)[:, tt, :])
                    hid_tiles.append(nsrc)
                wk_t = load_w(wk_d, "wk")
                wq_t = load_w(wq_d, "wq")
                wo_t = load_w(wo_d, "wo")

                # ---- V projection (half 0), hid transposes, V proj half 1 ----
                def vproj_half(half):
                    for ti in range(8):
                        pp = psmm.tile([P, 512], F32, tag="pp")
                        for di in range(8):
                            nc.tensor.matmul(
                                pp[:],
                                kvT[:, di, ti * P:(ti + 1) * P],
                                wv_t[:, di, half * 512:(half + 1) * 512],
                                start=(di == 0), stop=(di == 7),
                            )
                        dst = v65[:].rearrange(
                            "p t (h x) -> p t h x", x=65)[
                            :, ti, half * 8:(half + 1) * 8, 0:64]
                        nc.vector.tensor_tensor(
                            dst, pp[:],
                            vbB[:, half * 512:(half + 1) * 512], add)

                vproj_half(0)
                for tt in range(4):
                    nsrc = hid_tiles[tt]
                    for dhalf in range(2):
                        tp = pstp.tile([P, 512], BF16, tag="tp")
                        for q in range(4):
                            di = dhalf * 4 + q
                            nc.tensor.transpose(
                                tp[:, q * P:(q + 1) * P],
                                nsrc[:, di * P:(di + 1) * P],
                                ident[:],
                            )
                        dst_ap = hidT[:, dhalf * 4:dhalf * 4 + 4,
                                      tt * P:(tt + 1) * P]
                        src_ap = tp[:].rearrange("p (q x) -> p q x", q=4)
                        if dhalf == 0:
                            nc.scalar.activation(dst_ap, src_ap, Ident)
                        else:
                            nc.vector.tensor_copy(dst_ap, src_ap)
                vproj_half(1)

            # ---- main loop: per head-pair K/Q projection + attention ----
            with (
                tc.tile_pool(name="attn", bufs=2) as attnp,
                tc.tile_pool(name="rb", bufs=2) as rbp,
                tc.tile_pool(name="pssc2", bufs=2, space="PSUM") as pssc2,
                tc.tile_pool(name="psctx", bufs=2, space="PSUM") as psctx,
            ):
                def emit_kproj(hp, nk):
                    pp = psmm.tile([P, 512], F32, tag="pp",
                                   name=f"ppk{hp}_{nk}")
                    for di in range(8):
                        nc.tensor.matmul(
                            pp[:],
                            wk_t[:, di, hp * P:(hp + 1) * P],
                            kvT[:, di, nk * 512:(nk + 1) * 512],
                            start=(di == 0), stop=(di == 7),
                        )
                    nc.vector.tensor_scalar(
                        KT[:, hp, nk * 512:(nk + 1) * 512], pp[:],
                        kb_sb[:, hp:hp + 1], None, add)

                def emit_qproj(hp):
                    pq = psmm.tile([P, 512], F32, tag="pp", name=f"ppq{hp}")
                    for di in range(8):
                        nc.tensor.matmul(
                            pq[:],
                            wq_t[:, di, hp * P:(hp + 1) * P],
                            hidT[:, di, :],
                            start=(di == 0), stop=(di == 7),
                        )
                    nc.vector.tensor_scalar(qT[:, hp, :], pq[:],
                                            qb_sb[:, hp:hp + 1], None, add)

                def emit_scores(hp, t):
                    sc2 = pssc2.tile([P, 1024], F32, tag="sc2",
                                     name=f"sc2_{hp}_{t}")
                    for hh in range(2):
                        lo = 64 * hh
                        nc.tensor.matmul(
                            sc2[:, hh * 512:(hh + 1) * 512],
                            KT[lo:lo + 64, hp, t * P:(t + 1) * P],
                            qT[lo:lo + 64, hp, :],
                            start=True, stop=True,
                        )
                    at2 = attnp.tile([P, 1024], BF16, tag="at")
                    nc.scalar.activation(at2[:], sc2[:], Exp)
                    return at2

                def emit_attnv(hp, t, at2, ctx_ps):
                    for hh in range(2):
                        h = 2 * hp + hh
                        nc.tensor.matmul(
                            ctx_ps[hh][:],
                            v65[:, t, h * 65:(h + 1) * 65],
                            at2[:, hh * 512:(hh + 1) * 512],
                            start=(t == 0), stop=(t == 7),
                        )

                def emit_norm(hp, ctx_ps):
                    # ctx_ps[hh]: rows 0..63 unnormalized ctx^T, row 64 sums
                    for hh in range(2):
                        srow = rbp.tile([1, 512], F32, tag="srow",
                                        name=f"srow{hp}_{hh}")
                        nc.vector.tensor_copy(srow[:], ctx_ps[hh][64:65, :])
                        rcp = rbp.tile([1, 512], F32, tag="rcp",
                                       name=f"rcp{hp}_{hh}")
                        # custom-DVE ops misread partition-offset PSUM slices
                        # (microtest: 97% err) — stage the sums row in SBUF
                        nc.vector.reciprocal_approx_fast(rcp[:], srow[:])
                        bc = rbp.tile([P, 512], F32, tag="bc",
                                      name=f"bc{hp}_{hh}")
                        nc.gpsimd.partition_broadcast(bc[:], rcp[:])
                        lo = 64 * hh
                        nc.vector.tensor_tensor(
                            ctxT[lo:lo + 64, hp, :], ctx_ps[hh][0:64, :],
                            bc[lo:lo + 64, :], mult)

                emit_kproj(0, 0)
                emit_kproj(0, 1)
                emit_qproj(0)

                for hp in range(8):
                    nxt = hp + 1
                    ctx_ps = [psctx.tile([65, 512], F32, tag="ctx",
                                         name=f"ctx{hp}_{i}")
                              for i in range(2)]
                    # software pipeline: attnV lags scores by one kv tile so
                    # the EXP of tile t runs under scores(t+1)/attnV(t-1)
                    at_prev = emit_scores(hp, 0)
                    for t in range(1, 8):
                        at_cur = emit_scores(hp, t)
                        emit_attnv(hp, t - 1, at_prev, ctx_ps)
                        at_prev = at_cur
                        if nxt < 8:
                            if t == 2:
                                emit_kproj(nxt, 0)
                            elif t == 4:
                                emit_kproj(nxt, 1)
                            elif t == 6:
                                emit_qproj(nxt)
                    emit_attnv(hp, 7, at_prev, ctx_ps)
                    emit_norm(hp, ctx_ps)

            # ---- epilogue: out projection ----
            for half in range(2):
                for mi in range(4):
                    po = psmm.tile([P, 512], F32, tag="pp")
                    for fj in range(8):
                        nc.tensor.matmul(
                            po[:],
                            ctxT[:, fj, mi * P:(mi + 1) * P],
                            wo_t[:, fj, half * 512:(half + 1) * 512],
                            start=(fj == 0), stop=(fj == 7),
                        )
                    ot = outp.tile([P, 512], F32, tag="ot")
                    nc.vector.tensor_tensor(
                        ot[:], po[:],
                        obB[:, half * 512:(half + 1) * 512], add)
                    nc.sync.dma_start(
                        out_s.ap().rearrange("(mm p) d -> p mm d", p=P)[
                            :, mi, half * 512:(half + 1) * 512],
                        ot[:])

    nc.compile()
    return nc


def _w_layout(w_t):
    # [D, D] weight (already transposed: rows = contraction dim) ->
    # [128, 8*D] bf16 with element (p, dd*D+o) = w_t[dd*128+p, o]
    return np.ascontiguousarray(
        w_t.reshape(8, P, D).transpose(1, 0, 2).reshape(P, 8 * D)
    ).astype(ml_dtypes.bfloat16)


def _prep_inputs(hidden_states, key_value_states, q_weight, q_bias,
                 kv_weight, kv_bias, out_weight, out_bias):
    f32 = np.float32
    bf16 = ml_dtypes.bfloat16
    hid = np.asarray(hidden_states, f32).reshape(B * LQ, D).astype(bf16)
    kv = np.asarray(key_value_states, f32).reshape(B * LK, D).astype(bf16)
    scale = f32(1.0 / 8.0)

    # de-interleave kv rows: row e <-> (h=e//128, j=(e%128)//64, d=e%64)
    e = np.arange(2 * D)
    kmask = (e % 128) < 64
    kidx, vidx = e[kmask], e[~kmask]
    kvw = np.asarray(kv_weight, f32)
    kvb = np.asarray(kv_bias, f32)

    shared = {
        "wq_d": _w_layout((np.asarray(q_weight, f32) * scale).T),
        "wk_d": _w_layout(np.ascontiguousarray(kvw[kidx].T)),
        "wv_d": _w_layout(np.ascontiguousarray(kvw[vidx].T)),
        "wo_d": _w_layout(np.asarray(out_weight, f32).T),
        "qb": np.ascontiguousarray(np.asarray(q_bias, f32) * scale),
        "kb": np.ascontiguousarray(kvb[kidx]),
        "vb": np.ascontiguousarray(kvb[vidx]),
        "ob": np.ascontiguousarray(np.asarray(out_bias, f32)),
    }
    in_maps = []
    for c in range(NCORES):
        b = c // 2
        m = dict(shared)
        m["hid_s"] = np.ascontiguousarray(hid[c * TQ:(c + 1) * TQ])
        m["kv_s"] = np.ascontiguousarray(kv[b * LK:(b + 1) * LK])
        in_maps.append(m)
    return in_maps


def kernel(hidden_states, key_value_states, q_weight, q_bias,
           kv_weight, kv_bias, out_weight, out_bias, _trace=False):
    if "nc" not in _CACHE:
        _CACHE["nc"] = _build_core_program()
    nc = _CACHE["nc"]
    in_maps = _prep_inputs(hidden_states, key_value_states, q_weight, q_bias,
                           kv_weight, kv_bias, out_weight, out_bias)
    res = bass_utils.run_bass_kernel_spmd(
        nc, in_maps, core_ids=list(range(NCORES)), trace=_trace)
    _CACHE["last_result"] = res
    out = np.concatenate([r["out_s"] for r in res.results], axis=0)
    return out.reshape(B, LQ, D)
